# revision 12
# baseline (speedup 1.0000x reference)
"""Trainium2 Bass kernel for GNN message passing:
    out[i] = sum_{e: dst[e]==i} x[src[e]]     (x: [N, 64] f32, edge_index: [2, E] int)

Strategy (graph-partitioned node sharding, 8 cores):
  * Host sorts edges by destination and shards the destination-node space
    across the 8 cores (N/8 nodes per core, replicated x). Within each
    128-node destination tile, edges are bucketed by source block (25000
    rows, so block-local indices fit int16) and padded to 128-edge chunks.
  * x is repacked as [N, 128] bf16 rows: [bf16(x) | bf16(x - bf16(x))]
    (hi|lo split). One 256 B-row gather fetches both halves; one bf16
    matmul per chunk scatter-adds hi and lo into separate PSUM columns;
    they are summed at evacuation. This keeps ~1e-5 relative accuracy
    while running the PE at bf16 speed.
  * Per core, per supertile of 8 destination tiles (one [128, 1024] f32
    PSUM buffer = 2 banks, 8 tile slices):
      - dma_gather (GPSIMD ant instruction) fetches packed rows for up to
        64 chunks per call from one source block,
      - VectorE builds a [128, 128] bf16 one-hot per chunk
        (iota == local_dst; padded edges use local_dst = -1),
      - TensorE: psum[:, tile*128:+128] += onehot.T @ msgs (PSUM
        accumulates all chunks of a tile, duplicate-safe),
      - VectorE merges hi+lo into an SBUF staging buffer per tile.
  * Each core stores its [N/8, 64] f32 slice with one DMA; the host
    concatenates the 8 slices. No collectives.
"""

import numpy as np
import ml_dtypes

import concourse.bacc as bacc
import concourse.bass as bass
import concourse.mybir as mybir
import concourse.tile as tile
from concourse.bass_utils import run_bass_kernel_spmd

P = 128
F32 = mybir.dt.float32
BF16 = mybir.dt.bfloat16
I16 = mybir.dt.int16
I32 = mybir.dt.int32
BF = ml_dtypes.bfloat16

# Full-problem constants (hardcoded per harness contract).
N_NODES = 100000
DIM = 64
N_CORES = 8
SRC_BLOCK = 25000        # int16-safe source block
CHUNKS_PER_CALL = 64     # max 128-edge chunks per dma_gather call
SUPERTILE = 8            # dst tiles per PSUM buffer (one bank each)
SINGLE_PACKET = False    # >64-descriptor packets are out of spec; split packets


def _prep(edge_index, n_nodes, n_cores, block, w, stile=SUPERTILE):
    npc = n_nodes // n_cores
    tiles = -(-npc // P)
    nblocks = -(-n_nodes // block)

    dst = np.asarray(edge_index[0]).astype(np.int64)
    src = np.asarray(edge_index[1]).astype(np.int64)

    k_of = dst // npc
    t_of = (dst - k_of * npc) // P
    b_of = src // block
    seg = (k_of * tiles + t_of) * nblocks + b_of
    order = np.argsort(seg, kind="stable")
    dst_s = dst[order]
    src_s = src[order]
    seg_s = seg[order]

    counts = np.bincount(
        seg_s, minlength=n_cores * tiles * nblocks
    ).reshape(n_cores, tiles, nblocks)
    c_tb = (-(-counts // P)).max(axis=0)  # [tiles, nblocks] union chunk counts
    assert (counts <= c_tb[None] * P).all()

    n_super = -(-tiles // stile)
    # chunk order: for s, for b, for t in s
    chunk_tile = []      # global tile id per chunk
    chunk_block = []
    bucket_start = {}    # (t, b) -> first chunk index
    for s in range(n_super):
        ts = range(s * stile, min((s + 1) * stile, tiles))
        for b in range(nblocks):
            for t in ts:
                c = int(c_tb[t, b])
                if c == 0:
                    continue
                bucket_start[(t, b)] = len(chunk_tile)
                chunk_tile += [t] * c
                chunk_block += [b] * c
    ch = len(chunk_tile)
    chunk_tile = np.array(chunk_tile)
    chunk_block = np.array(chunk_block)
    chunk_super = chunk_tile // stile

    # per-tile first/last chunk in chunk order
    chunk_first = np.zeros(ch, dtype=bool)
    chunk_last = np.zeros(ch, dtype=bool)
    seen = set()
    for ci in range(ch):
        t = int(chunk_tile[ci])
        if t not in seen:
            seen.add(t)
            chunk_first[ci] = True
    seen = set()
    for ci in range(ch - 1, -1, -1):
        t = int(chunk_tile[ci])
        if t not in seen:
            seen.add(t)
            chunk_last[ci] = True
    tile_has_chunks = np.zeros(tiles, dtype=bool)
    tile_has_chunks[np.unique(chunk_tile)] = True

    # calls: split maximal same-(super, block) chunk runs into <= w pieces
    calls = []  # (block, c0, csize)
    c0 = 0
    for ci in range(1, ch + 1):
        if (
            ci == ch
            or chunk_block[ci] != chunk_block[c0]
            or chunk_super[ci] != chunk_super[c0]
        ):
            start = c0
            while start < ci:
                csize = min(w, ci - start)
                calls.append((int(chunk_block[c0]), start, csize))
                start += csize
            c0 = ci

    # per-core idx / ldst streams
    core_starts = np.searchsorted(seg_s, np.arange(n_cores) * tiles * nblocks)
    idx_flat = np.zeros((n_cores, ch * P), np.int16)
    ldst_flat = np.full((n_cores, ch * P), -1.0, np.float32)
    for k in range(n_cores):
        e = int(core_starts[k])
        for t in range(tiles):
            for b in range(nblocks):
                cnt = int(counts[k, t, b])
                if cnt == 0:
                    continue
                pos = bucket_start[(t, b)] * P
                idx_flat[k, pos : pos + cnt] = (
                    src_s[e : e + cnt] - b * block
                ).astype(np.int16)
                ldst_flat[k, pos : pos + cnt] = (
                    dst_s[e : e + cnt] - (k * npc + t * P)
                ).astype(np.float32)
                e += cnt

    # idx wrap-16 layout + replicate to the 8 gpsimd cores: [128, ch*8]
    idx_all = np.ascontiguousarray(
        np.tile(idx_flat.reshape(n_cores, ch * 8, 16).transpose(0, 2, 1), (1, 8, 1))
    )
    # ldst: [128, ch], [p, ci] = local dst of edge ci*128+p
    ldst_all = np.ascontiguousarray(
        ldst_flat.reshape(n_cores, ch, P).transpose(0, 2, 1)
    )

    return dict(
        npc=npc,
        tiles=tiles,
        nblocks=nblocks,
        n_super=n_super,
        stile=stile,
        ch=ch,
        calls=calls,
        chunk_tile=chunk_tile,
        chunk_super=chunk_super,
        chunk_first=chunk_first,
        chunk_last=chunk_last,
        tile_has_chunks=tile_has_chunks,
        idx=idx_all,
        ldst=ldst_all,
    )


def _pack_x(x):
    """[N, D] f32 -> [N, 2D] bf16 rows: [hi | lo]."""
    x = np.asarray(x, np.float32)
    hi = x.astype(BF)
    lo = (x - hi.astype(np.float32)).astype(BF)
    return np.ascontiguousarray(np.concatenate([hi, lo], axis=1))


def _build(n_nodes, dim, block, w, sched):
    tiles = sched["tiles"]
    stile = sched["stile"]
    n_super = sched["n_super"]
    ch = sched["ch"]
    calls = sched["calls"]
    chunk_tile = sched["chunk_tile"]
    chunk_first = sched["chunk_first"]
    chunk_last = sched["chunk_last"]
    tile_has = sched["tile_has_chunks"]
    out_pad = tiles * P
    elem = 2 * dim  # packed bf16 row length

    nc = bacc.Bacc("TRN2", target_bir_lowering=False, debug=False)
    x_t = nc.dram_tensor("xpack", [n_nodes, elem], BF16, kind="ExternalInput")
    idx_t = nc.dram_tensor("idx", [P, ch * 8], I16, kind="ExternalInput")
    ldst_t = nc.dram_tensor("ldst", [P, ch], F32, kind="ExternalInput")
    out_t = nc.dram_tensor("out", [out_pad, dim], F32, kind="ExternalOutput")

    with tile.TileContext(nc) as tc:
        with (
            tc.tile_pool(name="const", bufs=1) as const_pool,
            tc.tile_pool(name="meta", bufs=4) as meta_pool,
            tc.tile_pool(name="gather", bufs=3) as gather_pool,
            tc.tile_pool(name="oh", bufs=8) as oh_pool,
            tc.tile_pool(name="stage", bufs=1) as stage_pool,
            tc.tile_pool(name="psum", bufs=8, space="PSUM") as psum_pool,
        ):
            iota_i = const_pool.tile([P, P], I32)
            nc.gpsimd.iota(iota_i[:], pattern=[[1, P]], base=0, channel_multiplier=0)
            iota_b = const_pool.tile([P, P], BF16)
            nc.vector.tensor_copy(iota_b[:], iota_i[:])

            stage = stage_pool.tile([P, tiles * dim], F32)
            nc.vector.memset(stage[:], 0.0)

            # calls grouped by supertile; one PSUM bank per destination tile
            call_idx = 0
            psums = {}
            for s in range(n_super):
                ts = list(range(s * stile, min((s + 1) * stile, tiles)))
                while call_idx < len(calls):
                    b, c0, csize = calls[call_idx]
                    if int(sched["chunk_super"][c0]) != s:
                        break
                    call_idx += 1
                    idx_tile = meta_pool.tile([P, w * 8], I16, tag="idx")
                    nc.sync.dma_start(
                        idx_tile[:, : csize * 8],
                        idx_t[:, c0 * 8 : (c0 + csize) * 8],
                    )
                    ldst_tile = meta_pool.tile([P, w], F32, tag="ldst")
                    nc.sync.dma_start(
                        ldst_tile[:, :csize], ldst_t[:, c0 : c0 + csize]
                    )
                    msgs = gather_pool.tile([P, w, elem], BF16)
                    nc.gpsimd.dma_gather(
                        out_ap=msgs[:, :csize, :],
                        in_ap=x_t[b * block : min((b + 1) * block, n_nodes), :],
                        idxs_ap=idx_tile[:, : csize * 8],
                        num_idxs=csize * P,
                        num_idxs_reg=csize * P,
                        elem_size=elem,
                        single_packet=SINGLE_PACKET,
                    )
                    for j in range(csize):
                        ci = c0 + j
                        t = int(chunk_tile[ci])
                        onehot = oh_pool.tile([P, P], BF16)
                        nc.vector.tensor_scalar(
                            onehot[:],
                            iota_b[:],
                            ldst_tile[:, j : j + 1],
                            None,
                            op0=mybir.AluOpType.is_equal,
                        )
                        if chunk_first[ci]:
                            psums[t] = psum_pool.tile(
                                [P, elem], F32, tag="ps", name=f"ps{t}"
                            )
                        nc.tensor.matmul(
                            psums[t][:, :],
                            lhsT=onehot[:],
                            rhs=msgs[:, j, :],
                            start=bool(chunk_first[ci]),
                            stop=bool(chunk_last[ci]),
                        )
                # evacuate: stage[:, t*dim:+dim] = psum_hi + psum_lo
                for t in ts:
                    if not tile_has[t]:
                        continue
                    ps = psums.pop(t)
                    nc.scalar.copy(stage[:, t * dim : (t + 1) * dim], ps[:, :dim])
                    nc.vector.tensor_tensor(
                        out=stage[:, t * dim : (t + 1) * dim],
                        in0=stage[:, t * dim : (t + 1) * dim],
                        in1=ps[:, dim:],
                        op=mybir.AluOpType.add,
                    )

            out_view = out_t[:, :].rearrange("(t p) d -> p t d", p=P)
            nc.sync.dma_start(out_view, stage[:])

    nc.compile()
    return nc


def _run(x, edge_index, n_nodes, dim, n_cores, block, w, **run_kwargs):
    sched = _prep(edge_index, n_nodes, n_cores, block, w)
    xp = _pack_x(x)
    nc = _build(n_nodes, dim, block, w, sched)
    in_maps = [
        {"xpack": xp, "idx": sched["idx"][k], "ldst": sched["ldst"][k]}
        for k in range(n_cores)
    ]
    res = run_bass_kernel_spmd(
        nc, in_maps, core_ids=list(range(n_cores)), **run_kwargs
    )
    npc = sched["npc"]
    out = np.concatenate(
        [res.results[k]["out"][:npc] for k in range(n_cores)], axis=0
    )
    return out, res


def kernel(x, edge_index):
    out, _ = _run(
        x, edge_index, N_NODES, DIM, N_CORES, SRC_BLOCK, CHUNKS_PER_CALL
    )
    return out


# revision 16
# speedup vs baseline: 1.0167x; 1.0167x over previous
"""Trainium2 Bass kernel for GNN message passing:
    out[i] = sum_{e: dst[e]==i} x[src[e]]     (x: [N, 64] f32, edge_index: [2, E] int)

Strategy (graph-partitioned node sharding, 8 cores):
  * Host sorts edges by destination and shards the destination-node space
    across the 8 cores (N/8 nodes per core, replicated x). Within each
    128-node destination tile, edges are bucketed by source block (25000
    rows, so block-local indices fit int16) and padded to 128-edge chunks.
  * x is repacked as [N, 128] bf16 rows: [bf16(x) | bf16(x - bf16(x))]
    (hi|lo split). One 256 B-row gather fetches both halves; one bf16
    matmul per chunk scatter-adds hi and lo into separate PSUM columns;
    they are summed at evacuation. This keeps ~1e-5 relative accuracy
    while running the PE at bf16 speed.
  * Per core, per supertile of 8 destination tiles (one [128, 1024] f32
    PSUM buffer = 2 banks, 8 tile slices):
      - dma_gather (GPSIMD ant instruction) fetches packed rows for up to
        64 chunks per call from one source block,
      - VectorE builds a [128, 128] bf16 one-hot per chunk
        (iota == local_dst; padded edges use local_dst = -1),
      - TensorE: psum[:, tile*128:+128] += onehot.T @ msgs (PSUM
        accumulates all chunks of a tile, duplicate-safe),
      - VectorE merges hi+lo into an SBUF staging buffer per tile.
  * Each core stores its [N/8, 64] f32 slice with one DMA; the host
    concatenates the 8 slices. No collectives.
"""

import numpy as np
import ml_dtypes

import concourse.bacc as bacc
import concourse.bass as bass
import concourse.mybir as mybir
import concourse.tile as tile
from concourse.bass_utils import run_bass_kernel_spmd

P = 128
F32 = mybir.dt.float32
BF16 = mybir.dt.bfloat16
I16 = mybir.dt.int16
I32 = mybir.dt.int32
BF = ml_dtypes.bfloat16

# Full-problem constants (hardcoded per harness contract).
N_NODES = 100000
DIM = 64
N_CORES = 8
SRC_BLOCK = 25000        # int16-safe source block
CHUNKS_PER_CALL = 8      # max chunks per dma_gather call: single_packet
                         # packets cap at 64 data descriptors (+1 sem)
SUPERTILE = 8            # dst tiles per PSUM buffer (one bank each)
SINGLE_PACKET = True     # one packet per call; ring descs/call = 8*W+1,
                         # keep (8*W+1)*inflight <= 1024-slot carveout


def _prep(edge_index, n_nodes, n_cores, block, w, stile=SUPERTILE):
    npc = n_nodes // n_cores
    tiles = -(-npc // P)
    nblocks = -(-n_nodes // block)

    dst = np.asarray(edge_index[0]).astype(np.int64)
    src = np.asarray(edge_index[1]).astype(np.int64)

    k_of = dst // npc
    t_of = (dst - k_of * npc) // P
    b_of = src // block
    seg = (k_of * tiles + t_of) * nblocks + b_of
    order = np.argsort(seg, kind="stable")
    dst_s = dst[order]
    src_s = src[order]
    seg_s = seg[order]

    counts = np.bincount(
        seg_s, minlength=n_cores * tiles * nblocks
    ).reshape(n_cores, tiles, nblocks)
    c_tb = (-(-counts // P)).max(axis=0)  # [tiles, nblocks] union chunk counts
    assert (counts <= c_tb[None] * P).all()

    n_super = -(-tiles // stile)
    # chunk order: for s, for b, for t in s
    chunk_tile = []      # global tile id per chunk
    chunk_block = []
    bucket_start = {}    # (t, b) -> first chunk index
    for s in range(n_super):
        ts = range(s * stile, min((s + 1) * stile, tiles))
        for b in range(nblocks):
            for t in ts:
                c = int(c_tb[t, b])
                if c == 0:
                    continue
                bucket_start[(t, b)] = len(chunk_tile)
                chunk_tile += [t] * c
                chunk_block += [b] * c
    ch = len(chunk_tile)
    chunk_tile = np.array(chunk_tile)
    chunk_block = np.array(chunk_block)
    chunk_super = chunk_tile // stile

    # per-tile first/last chunk in chunk order
    chunk_first = np.zeros(ch, dtype=bool)
    chunk_last = np.zeros(ch, dtype=bool)
    seen = set()
    for ci in range(ch):
        t = int(chunk_tile[ci])
        if t not in seen:
            seen.add(t)
            chunk_first[ci] = True
    seen = set()
    for ci in range(ch - 1, -1, -1):
        t = int(chunk_tile[ci])
        if t not in seen:
            seen.add(t)
            chunk_last[ci] = True
    tile_has_chunks = np.zeros(tiles, dtype=bool)
    tile_has_chunks[np.unique(chunk_tile)] = True

    # calls: split maximal same-(super, block) chunk runs into <= w pieces
    calls = []  # (block, c0, csize)
    c0 = 0
    for ci in range(1, ch + 1):
        if (
            ci == ch
            or chunk_block[ci] != chunk_block[c0]
            or chunk_super[ci] != chunk_super[c0]
        ):
            start = c0
            while start < ci:
                csize = min(w, ci - start)
                calls.append((int(chunk_block[c0]), start, csize))
                start += csize
            c0 = ci

    # per-core idx / ldst streams
    core_starts = np.searchsorted(seg_s, np.arange(n_cores) * tiles * nblocks)
    idx_flat = np.zeros((n_cores, ch * P), np.int16)
    ldst_flat = np.full((n_cores, ch * P), -1.0, BF)
    for k in range(n_cores):
        e = int(core_starts[k])
        for t in range(tiles):
            for b in range(nblocks):
                cnt = int(counts[k, t, b])
                if cnt == 0:
                    continue
                pos = bucket_start[(t, b)] * P
                idx_flat[k, pos : pos + cnt] = (
                    src_s[e : e + cnt] - b * block
                ).astype(np.int16)
                ldst_flat[k, pos : pos + cnt] = (
                    dst_s[e : e + cnt] - (k * npc + t * P)
                ).astype(BF)
                e += cnt

    # idx wrap-16 layout + replicate to the 8 gpsimd cores: [128, ch*8]
    idx_all = np.ascontiguousarray(
        np.tile(idx_flat.reshape(n_cores, ch * 8, 16).transpose(0, 2, 1), (1, 8, 1))
    )
    # ldst: [128, ch], [p, ci] = local dst of edge ci*128+p
    ldst_all = np.ascontiguousarray(
        ldst_flat.reshape(n_cores, ch, P).transpose(0, 2, 1)
    )

    return dict(
        npc=npc,
        tiles=tiles,
        nblocks=nblocks,
        n_super=n_super,
        stile=stile,
        ch=ch,
        calls=calls,
        chunk_tile=chunk_tile,
        chunk_super=chunk_super,
        chunk_first=chunk_first,
        chunk_last=chunk_last,
        tile_has_chunks=tile_has_chunks,
        idx=idx_all,
        ldst=ldst_all,
    )


def _pack_x(x):
    """[N, D] f32 -> [N, 2D] bf16 rows: [hi | lo]."""
    x = np.asarray(x, np.float32)
    hi = x.astype(BF)
    lo = (x - hi.astype(np.float32)).astype(BF)
    return np.ascontiguousarray(np.concatenate([hi, lo], axis=1))


def _build(n_nodes, dim, block, w, sched):
    tiles = sched["tiles"]
    stile = sched["stile"]
    n_super = sched["n_super"]
    ch = sched["ch"]
    calls = sched["calls"]
    chunk_tile = sched["chunk_tile"]
    chunk_first = sched["chunk_first"]
    chunk_last = sched["chunk_last"]
    tile_has = sched["tile_has_chunks"]
    out_pad = tiles * P
    elem = 2 * dim  # packed bf16 row length

    nc = bacc.Bacc("TRN2", target_bir_lowering=False, debug=False)
    x_t = nc.dram_tensor("xpack", [n_nodes, elem], BF16, kind="ExternalInput")
    idx_t = nc.dram_tensor("idx", [P, ch * 8], I16, kind="ExternalInput")
    ldst_t = nc.dram_tensor("ldst", [P, ch], BF16, kind="ExternalInput")
    out_t = nc.dram_tensor("out", [out_pad, dim], F32, kind="ExternalOutput")

    with tile.TileContext(nc) as tc:
        with (
            tc.tile_pool(name="const", bufs=1) as const_pool,
            tc.tile_pool(name="meta", bufs=4) as meta_pool,
            tc.tile_pool(name="gather", bufs=3) as gather_pool,
            tc.tile_pool(name="oh", bufs=8) as oh_pool,
            tc.tile_pool(name="stage", bufs=1) as stage_pool,
            tc.tile_pool(name="psum", bufs=8, space="PSUM") as psum_pool,
        ):
            iota_i = const_pool.tile([P, 4 * P], I32)
            nc.gpsimd.iota(
                iota_i[:], pattern=[[0, 4], [1, P]], base=0, channel_multiplier=0
            )
            iota_b = const_pool.tile([P, 4 * P], BF16)
            nc.vector.tensor_copy(iota_b[:], iota_i[:])

            stage = stage_pool.tile([P, tiles * dim], F32)
            nc.vector.memset(stage[:], 0.0)

            # calls grouped by supertile; one PSUM bank per destination tile
            call_idx = 0
            psums = {}
            for s in range(n_super):
                ts = list(range(s * stile, min((s + 1) * stile, tiles)))
                while call_idx < len(calls):
                    b, c0, csize = calls[call_idx]
                    if int(sched["chunk_super"][c0]) != s:
                        break
                    qn = 0
                    call_idx += 1
                    idx_tile = meta_pool.tile([P, w * 8], I16, tag="idx")
                    nc.sync.dma_start(
                        idx_tile[:, : csize * 8],
                        idx_t[:, c0 * 8 : (c0 + csize) * 8],
                    )
                    ldst_tile = meta_pool.tile([P, w], BF16, tag="ldst")
                    nc.sync.dma_start(
                        ldst_tile[:, :csize], ldst_t[:, c0 : c0 + csize]
                    )
                    msgs = gather_pool.tile([P, w, elem], BF16)
                    nc.gpsimd.dma_gather(
                        out_ap=msgs[:, :csize, :],
                        in_ap=x_t[b * block : min((b + 1) * block, n_nodes), :],
                        idxs_ap=idx_tile[:, : csize * 8],
                        num_idxs=csize * P,
                        num_idxs_reg=csize * P,
                        elem_size=elem,
                        single_packet=SINGLE_PACKET,
                        queue_num=qn,
                    )
                    for j0 in range(0, csize, 4):
                        g = min(4, csize - j0)
                        onehot = oh_pool.tile([P, 4 * P], BF16, name="oh", tag="oh")
                        lt = ldst_tile[:, j0 : j0 + g]
                        lt_b = bass.AP(lt.tensor, lt.offset, lt.ap + [[0, P]])
                        nc.vector.tensor_tensor(
                            out=onehot[:, : g * P].rearrange(
                                "p (g q) -> p g q", q=P
                            ),
                            in0=iota_b[:, : g * P].rearrange(
                                "p (g q) -> p g q", q=P
                            ),
                            in1=lt_b,
                            op=mybir.AluOpType.is_equal,
                        )
                        for jj in range(g):
                            ci = c0 + j0 + jj
                            t = int(chunk_tile[ci])
                            if chunk_first[ci]:
                                psums[t] = psum_pool.tile(
                                    [P, elem], F32, tag="ps", name=f"ps{t}"
                                )
                            nc.tensor.matmul(
                                psums[t][:, :],
                                lhsT=onehot[:, (jj) * P : (jj + 1) * P],
                                rhs=msgs[:, j0 + jj, :],
                                start=bool(chunk_first[ci]),
                                stop=bool(chunk_last[ci]),
                            )
                # evacuate: stage[:, t*dim:+dim] = psum_hi + psum_lo
                for t in ts:
                    if not tile_has[t]:
                        continue
                    ps = psums.pop(t)
                    nc.scalar.copy(stage[:, t * dim : (t + 1) * dim], ps[:, :dim])
                    nc.vector.tensor_tensor(
                        out=stage[:, t * dim : (t + 1) * dim],
                        in0=stage[:, t * dim : (t + 1) * dim],
                        in1=ps[:, dim:],
                        op=mybir.AluOpType.add,
                    )

            out_view = out_t[:, :].rearrange("(t p) d -> p t d", p=P)
            nc.sync.dma_start(out_view, stage[:])

    nc.compile()
    return nc


def _run(x, edge_index, n_nodes, dim, n_cores, block, w, **run_kwargs):
    sched = _prep(edge_index, n_nodes, n_cores, block, w)
    xp = _pack_x(x)
    nc = _build(n_nodes, dim, block, w, sched)
    in_maps = [
        {"xpack": xp, "idx": sched["idx"][k], "ldst": sched["ldst"][k]}
        for k in range(n_cores)
    ]
    res = run_bass_kernel_spmd(
        nc, in_maps, core_ids=list(range(n_cores)), **run_kwargs
    )
    npc = sched["npc"]
    out = np.concatenate(
        [res.results[k]["out"][:npc] for k in range(n_cores)], axis=0
    )
    return out, res


def kernel(x, edge_index):
    out, _ = _run(
        x, edge_index, N_NODES, DIM, N_CORES, SRC_BLOCK, CHUNKS_PER_CALL
    )
    return out


# revision 17
# speedup vs baseline: 1.0961x; 1.0781x over previous
"""Trainium2 Bass kernel for GNN message passing:
    out[i] = sum_{e: dst[e]==i} x[src[e]]     (x: [N, 64] f32, edge_index: [2, E] int)

Strategy (graph-partitioned node sharding, 8 cores):
  * Host sorts edges by destination and shards the destination-node space
    across the 8 cores (N/8 nodes per core, replicated x). Within each
    128-node destination tile, edges are bucketed by source block (25000
    rows, so block-local indices fit int16) and padded to 128-edge chunks.
  * x is repacked as [N, 128] bf16 rows: [bf16(x) | bf16(x - bf16(x))]
    (hi|lo split). One 256 B-row gather fetches both halves; one bf16
    matmul per chunk scatter-adds hi and lo into separate PSUM columns;
    they are summed at evacuation. This keeps ~1e-5 relative accuracy
    while running the PE at bf16 speed.
  * Per core, per supertile of 8 destination tiles (one [128, 1024] f32
    PSUM buffer = 2 banks, 8 tile slices):
      - dma_gather (GPSIMD ant instruction) fetches packed rows for up to
        64 chunks per call from one source block,
      - VectorE builds a [128, 128] bf16 one-hot per chunk
        (iota == local_dst; padded edges use local_dst = -1),
      - TensorE: psum[:, tile*128:+128] += onehot.T @ msgs (PSUM
        accumulates all chunks of a tile, duplicate-safe),
      - VectorE merges hi+lo into an SBUF staging buffer per tile.
  * Each core stores its [N/8, 64] f32 slice with one DMA; the host
    concatenates the 8 slices. No collectives.
"""

import numpy as np
import ml_dtypes

import concourse.bacc as bacc
import concourse.bass as bass
import concourse.mybir as mybir
import concourse.tile as tile
from concourse.bass_utils import run_bass_kernel_spmd

P = 128
F32 = mybir.dt.float32
BF16 = mybir.dt.bfloat16
I16 = mybir.dt.int16
I32 = mybir.dt.int32
BF = ml_dtypes.bfloat16

# Full-problem constants (hardcoded per harness contract).
N_NODES = 100000
DIM = 64
N_CORES = 8
SRC_BLOCK = 25000        # int16-safe source block
CHUNKS_PER_CALL = 48     # max chunks per dma_gather call; split packets
                         # (single_packet caps at 64 ring descriptors = 8 chunks)
SUPERTILE = 8            # dst tiles per PSUM buffer (one bank each)
SINGLE_PACKET = False    # ring descs/call = 16*W+2 must stay < 1024 carveout


def _prep(edge_index, n_nodes, n_cores, block, w, stile=SUPERTILE):
    npc = n_nodes // n_cores
    tiles = -(-npc // P)
    nblocks = -(-n_nodes // block)

    dst = np.asarray(edge_index[0]).astype(np.int64)
    src = np.asarray(edge_index[1]).astype(np.int64)

    k_of = dst // npc
    t_of = (dst - k_of * npc) // P
    b_of = src // block
    seg = (k_of * tiles + t_of) * nblocks + b_of
    order = np.argsort(seg, kind="stable")
    dst_s = dst[order]
    src_s = src[order]
    seg_s = seg[order]

    counts = np.bincount(
        seg_s, minlength=n_cores * tiles * nblocks
    ).reshape(n_cores, tiles, nblocks)
    c_tb = (-(-counts // P)).max(axis=0)  # [tiles, nblocks] union chunk counts
    assert (counts <= c_tb[None] * P).all()

    n_super = -(-tiles // stile)
    # chunk order: for s, for b, for t in s
    chunk_tile = []      # global tile id per chunk
    chunk_block = []
    bucket_start = {}    # (t, b) -> first chunk index
    for s in range(n_super):
        ts = range(s * stile, min((s + 1) * stile, tiles))
        for b in range(nblocks):
            for t in ts:
                c = int(c_tb[t, b])
                if c == 0:
                    continue
                bucket_start[(t, b)] = len(chunk_tile)
                chunk_tile += [t] * c
                chunk_block += [b] * c
    ch = len(chunk_tile)
    chunk_tile = np.array(chunk_tile)
    chunk_block = np.array(chunk_block)
    chunk_super = chunk_tile // stile

    # per-tile first/last chunk in chunk order
    chunk_first = np.zeros(ch, dtype=bool)
    chunk_last = np.zeros(ch, dtype=bool)
    seen = set()
    for ci in range(ch):
        t = int(chunk_tile[ci])
        if t not in seen:
            seen.add(t)
            chunk_first[ci] = True
    seen = set()
    for ci in range(ch - 1, -1, -1):
        t = int(chunk_tile[ci])
        if t not in seen:
            seen.add(t)
            chunk_last[ci] = True
    tile_has_chunks = np.zeros(tiles, dtype=bool)
    tile_has_chunks[np.unique(chunk_tile)] = True

    # calls: split maximal same-(super, block) chunk runs into <= w pieces
    calls = []  # (block, c0, csize)
    c0 = 0
    for ci in range(1, ch + 1):
        if (
            ci == ch
            or chunk_block[ci] != chunk_block[c0]
            or chunk_super[ci] != chunk_super[c0]
        ):
            start = c0
            while start < ci:
                csize = min(w, ci - start)
                calls.append((int(chunk_block[c0]), start, csize))
                start += csize
            c0 = ci

    # per-core idx / ldst streams
    core_starts = np.searchsorted(seg_s, np.arange(n_cores) * tiles * nblocks)
    idx_flat = np.zeros((n_cores, ch * P), np.int16)
    ldst_flat = np.full((n_cores, ch * P), -1.0, BF)
    for k in range(n_cores):
        e = int(core_starts[k])
        for t in range(tiles):
            for b in range(nblocks):
                cnt = int(counts[k, t, b])
                if cnt == 0:
                    continue
                pos = bucket_start[(t, b)] * P
                idx_flat[k, pos : pos + cnt] = (
                    src_s[e : e + cnt] - b * block
                ).astype(np.int16)
                ldst_flat[k, pos : pos + cnt] = (
                    dst_s[e : e + cnt] - (k * npc + t * P)
                ).astype(BF)
                e += cnt

    # idx wrap-16 layout + replicate to the 8 gpsimd cores: [128, ch*8]
    idx_all = np.ascontiguousarray(
        np.tile(idx_flat.reshape(n_cores, ch * 8, 16).transpose(0, 2, 1), (1, 8, 1))
    )
    # ldst: [128, ch], [p, ci] = local dst of edge ci*128+p
    ldst_all = np.ascontiguousarray(
        ldst_flat.reshape(n_cores, ch, P).transpose(0, 2, 1)
    )

    return dict(
        npc=npc,
        tiles=tiles,
        nblocks=nblocks,
        n_super=n_super,
        stile=stile,
        ch=ch,
        calls=calls,
        chunk_tile=chunk_tile,
        chunk_super=chunk_super,
        chunk_first=chunk_first,
        chunk_last=chunk_last,
        tile_has_chunks=tile_has_chunks,
        idx=idx_all,
        ldst=ldst_all,
    )


def _pack_x(x):
    """[N, D] f32 -> [N, 2D] bf16 rows: [hi | lo]."""
    x = np.asarray(x, np.float32)
    hi = x.astype(BF)
    lo = (x - hi.astype(np.float32)).astype(BF)
    return np.ascontiguousarray(np.concatenate([hi, lo], axis=1))


def _build(n_nodes, dim, block, w, sched):
    tiles = sched["tiles"]
    stile = sched["stile"]
    n_super = sched["n_super"]
    ch = sched["ch"]
    calls = sched["calls"]
    chunk_tile = sched["chunk_tile"]
    chunk_first = sched["chunk_first"]
    chunk_last = sched["chunk_last"]
    tile_has = sched["tile_has_chunks"]
    out_pad = tiles * P
    elem = 2 * dim  # packed bf16 row length

    nc = bacc.Bacc("TRN2", target_bir_lowering=False, debug=False)
    x_t = nc.dram_tensor("xpack", [n_nodes, elem], BF16, kind="ExternalInput")
    idx_t = nc.dram_tensor("idx", [P, ch * 8], I16, kind="ExternalInput")
    ldst_t = nc.dram_tensor("ldst", [P, ch], BF16, kind="ExternalInput")
    out_t = nc.dram_tensor("out", [out_pad, dim], F32, kind="ExternalOutput")

    with tile.TileContext(nc) as tc:
        with (
            tc.tile_pool(name="const", bufs=1) as const_pool,
            tc.tile_pool(name="meta", bufs=4) as meta_pool,
            tc.tile_pool(name="gather", bufs=3) as gather_pool,
            tc.tile_pool(name="oh", bufs=8) as oh_pool,
            tc.tile_pool(name="stage", bufs=1) as stage_pool,
            tc.tile_pool(name="psum", bufs=8, space="PSUM") as psum_pool,
        ):
            iota_i = const_pool.tile([P, 4 * P], I32)
            nc.gpsimd.iota(
                iota_i[:], pattern=[[0, 4], [1, P]], base=0, channel_multiplier=0
            )
            iota_b = const_pool.tile([P, 4 * P], BF16)
            nc.vector.tensor_copy(iota_b[:], iota_i[:])

            stage = stage_pool.tile([P, tiles * dim], F32)
            nc.vector.memset(stage[:], 0.0)

            # calls grouped by supertile; one PSUM bank per destination tile
            call_idx = 0
            psums = {}
            for s in range(n_super):
                ts = list(range(s * stile, min((s + 1) * stile, tiles)))
                while call_idx < len(calls):
                    b, c0, csize = calls[call_idx]
                    if int(sched["chunk_super"][c0]) != s:
                        break
                    qn = 0
                    call_idx += 1
                    idx_tile = meta_pool.tile([P, w * 8], I16, tag="idx")
                    nc.sync.dma_start(
                        idx_tile[:, : csize * 8],
                        idx_t[:, c0 * 8 : (c0 + csize) * 8],
                    )
                    ldst_tile = meta_pool.tile([P, w], BF16, tag="ldst")
                    nc.sync.dma_start(
                        ldst_tile[:, :csize], ldst_t[:, c0 : c0 + csize]
                    )
                    msgs = gather_pool.tile([P, w, elem], BF16)
                    nc.gpsimd.dma_gather(
                        out_ap=msgs[:, :csize, :],
                        in_ap=x_t[b * block : min((b + 1) * block, n_nodes), :],
                        idxs_ap=idx_tile[:, : csize * 8],
                        num_idxs=csize * P,
                        num_idxs_reg=csize * P,
                        elem_size=elem,
                        single_packet=SINGLE_PACKET,
                        queue_num=qn,
                    )
                    for j0 in range(0, csize, 4):
                        g = min(4, csize - j0)
                        onehot = oh_pool.tile([P, 4 * P], BF16, name="oh", tag="oh")
                        lt = ldst_tile[:, j0 : j0 + g]
                        lt_b = bass.AP(lt.tensor, lt.offset, lt.ap + [[0, P]])
                        nc.vector.tensor_tensor(
                            out=onehot[:, : g * P].rearrange(
                                "p (g q) -> p g q", q=P
                            ),
                            in0=iota_b[:, : g * P].rearrange(
                                "p (g q) -> p g q", q=P
                            ),
                            in1=lt_b,
                            op=mybir.AluOpType.is_equal,
                        )
                        for jj in range(g):
                            ci = c0 + j0 + jj
                            t = int(chunk_tile[ci])
                            if chunk_first[ci]:
                                psums[t] = psum_pool.tile(
                                    [P, elem], F32, tag="ps", name=f"ps{t}"
                                )
                            nc.tensor.matmul(
                                psums[t][:, :],
                                lhsT=onehot[:, (jj) * P : (jj + 1) * P],
                                rhs=msgs[:, j0 + jj, :],
                                start=bool(chunk_first[ci]),
                                stop=bool(chunk_last[ci]),
                            )
                # evacuate: stage[:, t*dim:+dim] = psum_hi + psum_lo
                for t in ts:
                    if not tile_has[t]:
                        continue
                    ps = psums.pop(t)
                    nc.scalar.copy(stage[:, t * dim : (t + 1) * dim], ps[:, :dim])
                    nc.vector.tensor_tensor(
                        out=stage[:, t * dim : (t + 1) * dim],
                        in0=stage[:, t * dim : (t + 1) * dim],
                        in1=ps[:, dim:],
                        op=mybir.AluOpType.add,
                    )

            out_view = out_t[:, :].rearrange("(t p) d -> p t d", p=P)
            nc.sync.dma_start(out_view, stage[:])

    nc.compile()
    return nc


def _run(x, edge_index, n_nodes, dim, n_cores, block, w, **run_kwargs):
    sched = _prep(edge_index, n_nodes, n_cores, block, w)
    xp = _pack_x(x)
    nc = _build(n_nodes, dim, block, w, sched)
    in_maps = [
        {"xpack": xp, "idx": sched["idx"][k], "ldst": sched["ldst"][k]}
        for k in range(n_cores)
    ]
    res = run_bass_kernel_spmd(
        nc, in_maps, core_ids=list(range(n_cores)), **run_kwargs
    )
    npc = sched["npc"]
    out = np.concatenate(
        [res.results[k]["out"][:npc] for k in range(n_cores)], axis=0
    )
    return out, res


def kernel(x, edge_index):
    out, _ = _run(
        x, edge_index, N_NODES, DIM, N_CORES, SRC_BLOCK, CHUNKS_PER_CALL
    )
    return out


# revision 18
# speedup vs baseline: 1.1787x; 1.0753x over previous
"""Trainium2 Bass kernel for GNN message passing:
    out[i] = sum_{e: dst[e]==i} x[src[e]]     (x: [N, 64] f32, edge_index: [2, E] int)

Strategy (graph-partitioned node sharding, 8 cores):
  * Host sorts edges by destination and shards the destination-node space
    across the 8 cores (N/8 nodes per core, replicated x). Within each
    128-node destination tile, edges are bucketed by source block (25000
    rows, so block-local indices fit int16) and padded to 128-edge chunks.
  * x is repacked as [N, 128] bf16 rows: [bf16(x) | bf16(x - bf16(x))]
    (hi|lo split). One 256 B-row gather fetches both halves; one bf16
    matmul per chunk scatter-adds hi and lo into separate PSUM columns;
    they are summed at evacuation. This keeps ~1e-5 relative accuracy
    while running the PE at bf16 speed.
  * Per core, per supertile of 8 destination tiles (one [128, 1024] f32
    PSUM buffer = 2 banks, 8 tile slices):
      - dma_gather (GPSIMD ant instruction) fetches packed rows for up to
        64 chunks per call from one source block,
      - VectorE builds a [128, 128] bf16 one-hot per chunk
        (iota == local_dst; padded edges use local_dst = -1),
      - TensorE: psum[:, tile*128:+128] += onehot.T @ msgs (PSUM
        accumulates all chunks of a tile, duplicate-safe),
      - VectorE merges hi+lo into an SBUF staging buffer per tile.
  * Each core stores its [N/8, 64] f32 slice with one DMA; the host
    concatenates the 8 slices. No collectives.
"""

import numpy as np
import ml_dtypes

import concourse.bacc as bacc
import concourse.bass as bass
import concourse.mybir as mybir
import concourse.tile as tile
from concourse.bass_utils import run_bass_kernel_spmd

P = 128
F32 = mybir.dt.float32
BF16 = mybir.dt.bfloat16
I16 = mybir.dt.int16
I32 = mybir.dt.int32
BF = ml_dtypes.bfloat16

# Full-problem constants (hardcoded per harness contract).
N_NODES = 100000
DIM = 64
N_CORES = 8
SRC_BLOCK = 25000        # int16-safe source block
CHUNKS_PER_CALL = 48     # max chunks per dma_gather call; split packets
                         # (single_packet caps at 64 ring descriptors = 8 chunks)
SUPERTILE = 8            # dst tiles per PSUM buffer (one bank each)
SINGLE_PACKET = False    # ring descs/call = 16*W+2 must stay < 1024 carveout


def _prep(edge_index, n_nodes, n_cores, block, w, stile=SUPERTILE):
    npc = n_nodes // n_cores
    tiles = -(-npc // P)
    nblocks = -(-n_nodes // block)

    dst = np.asarray(edge_index[0]).astype(np.int64)
    src = np.asarray(edge_index[1]).astype(np.int64)

    k_of = dst // npc
    t_of = (dst - k_of * npc) // P
    b_of = src // block
    seg = (k_of * tiles + t_of) * nblocks + b_of
    order = np.argsort(seg, kind="stable")
    dst_s = dst[order]
    src_s = src[order]
    seg_s = seg[order]

    counts = np.bincount(
        seg_s, minlength=n_cores * tiles * nblocks
    ).reshape(n_cores, tiles, nblocks)
    # Per-core tile->slot permutation: sort each core's tiles by edge count so
    # the union-max over cores aligns heavy tiles with heavy tiles. The host
    # un-permutes output rows afterward.
    perm = np.argsort(-counts.sum(axis=2), axis=1, kind="stable")  # [cores, tiles]
    counts = np.take_along_axis(counts, perm[:, :, None], axis=1)  # slot-aligned
    c_tb = (-(-counts // P)).max(axis=0)  # [slots, nblocks] union chunk counts
    assert (counts <= c_tb[None] * P).all()

    n_super = -(-tiles // stile)
    # chunk order: for s, for b, for t in s
    chunk_tile = []      # global tile id per chunk
    chunk_block = []
    bucket_start = {}    # (t, b) -> first chunk index
    for s in range(n_super):
        ts = range(s * stile, min((s + 1) * stile, tiles))
        for b in range(nblocks):
            for t in ts:
                c = int(c_tb[t, b])
                if c == 0:
                    continue
                bucket_start[(t, b)] = len(chunk_tile)
                chunk_tile += [t] * c
                chunk_block += [b] * c
    ch = len(chunk_tile)
    chunk_tile = np.array(chunk_tile)
    chunk_block = np.array(chunk_block)
    chunk_super = chunk_tile // stile

    # per-tile first/last chunk in chunk order
    chunk_first = np.zeros(ch, dtype=bool)
    chunk_last = np.zeros(ch, dtype=bool)
    seen = set()
    for ci in range(ch):
        t = int(chunk_tile[ci])
        if t not in seen:
            seen.add(t)
            chunk_first[ci] = True
    seen = set()
    for ci in range(ch - 1, -1, -1):
        t = int(chunk_tile[ci])
        if t not in seen:
            seen.add(t)
            chunk_last[ci] = True
    tile_has_chunks = np.zeros(tiles, dtype=bool)
    tile_has_chunks[np.unique(chunk_tile)] = True

    # calls: split maximal same-(super, block) chunk runs into <= w pieces
    calls = []  # (block, c0, csize)
    c0 = 0
    for ci in range(1, ch + 1):
        if (
            ci == ch
            or chunk_block[ci] != chunk_block[c0]
            or chunk_super[ci] != chunk_super[c0]
        ):
            start = c0
            while start < ci:
                csize = min(w, ci - start)
                calls.append((int(chunk_block[c0]), start, csize))
                start += csize
            c0 = ci

    # per-core idx / ldst streams
    core_starts = np.searchsorted(seg_s, np.arange(n_cores) * tiles * nblocks)
    idx_flat = np.zeros((n_cores, ch * P), np.int16)
    ldst_flat = np.full((n_cores, ch * P), -1.0, BF)
    # per (core, true tile, block) edge-stream offsets
    tb_counts0 = np.bincount(
        seg_s, minlength=n_cores * tiles * nblocks
    ).reshape(n_cores, tiles * nblocks)
    tb_starts = np.concatenate(
        [np.zeros((n_cores, 1), np.int64), np.cumsum(tb_counts0, axis=1)], axis=1
    )
    for k in range(n_cores):
        e0 = int(core_starts[k])
        for t in range(tiles):  # slot
            tt = int(perm[k, t])  # true tile
            for b in range(nblocks):
                cnt = int(counts[k, t, b])
                if cnt == 0:
                    continue
                e = e0 + int(tb_starts[k, tt * nblocks + b])
                pos = bucket_start[(t, b)] * P
                idx_flat[k, pos : pos + cnt] = (
                    src_s[e : e + cnt] - b * block
                ).astype(np.int16)
                ldst_flat[k, pos : pos + cnt] = (
                    dst_s[e : e + cnt] - (k * npc + tt * P)
                ).astype(BF)

    # idx wrap-16 layout + replicate to the 8 gpsimd cores: [128, ch*8]
    idx_all = np.ascontiguousarray(
        np.tile(idx_flat.reshape(n_cores, ch * 8, 16).transpose(0, 2, 1), (1, 8, 1))
    )
    # ldst: [128, ch], [p, ci] = local dst of edge ci*128+p
    ldst_all = np.ascontiguousarray(
        ldst_flat.reshape(n_cores, ch, P).transpose(0, 2, 1)
    )

    return dict(
        npc=npc,
        tiles=tiles,
        nblocks=nblocks,
        n_super=n_super,
        stile=stile,
        ch=ch,
        calls=calls,
        chunk_tile=chunk_tile,
        chunk_super=chunk_super,
        chunk_first=chunk_first,
        chunk_last=chunk_last,
        tile_has_chunks=tile_has_chunks,
        idx=idx_all,
        ldst=ldst_all,
        perm=perm,
    )


def _pack_x(x):
    """[N, D] f32 -> [N, 2D] bf16 rows: [hi | lo]."""
    x = np.asarray(x, np.float32)
    hi = x.astype(BF)
    lo = (x - hi.astype(np.float32)).astype(BF)
    return np.ascontiguousarray(np.concatenate([hi, lo], axis=1))


def _build(n_nodes, dim, block, w, sched):
    tiles = sched["tiles"]
    stile = sched["stile"]
    n_super = sched["n_super"]
    ch = sched["ch"]
    calls = sched["calls"]
    chunk_tile = sched["chunk_tile"]
    chunk_first = sched["chunk_first"]
    chunk_last = sched["chunk_last"]
    tile_has = sched["tile_has_chunks"]
    out_pad = tiles * P
    elem = 2 * dim  # packed bf16 row length

    nc = bacc.Bacc("TRN2", target_bir_lowering=False, debug=False)
    x_t = nc.dram_tensor("xpack", [n_nodes, elem], BF16, kind="ExternalInput")
    idx_t = nc.dram_tensor("idx", [P, ch * 8], I16, kind="ExternalInput")
    ldst_t = nc.dram_tensor("ldst", [P, ch], BF16, kind="ExternalInput")
    out_t = nc.dram_tensor("out", [out_pad, dim], F32, kind="ExternalOutput")

    with tile.TileContext(nc) as tc:
        with (
            tc.tile_pool(name="const", bufs=1) as const_pool,
            tc.tile_pool(name="meta", bufs=4) as meta_pool,
            tc.tile_pool(name="gather", bufs=3) as gather_pool,
            tc.tile_pool(name="oh", bufs=8) as oh_pool,
            tc.tile_pool(name="stage", bufs=1) as stage_pool,
            tc.tile_pool(name="psum", bufs=8, space="PSUM") as psum_pool,
        ):
            iota_i = const_pool.tile([P, 4 * P], I32)
            nc.gpsimd.iota(
                iota_i[:], pattern=[[0, 4], [1, P]], base=0, channel_multiplier=0
            )
            iota_b = const_pool.tile([P, 4 * P], BF16)
            nc.vector.tensor_copy(iota_b[:], iota_i[:])

            stage = stage_pool.tile([P, tiles * dim], F32)
            nc.vector.memset(stage[:], 0.0)

            # calls grouped by supertile; one PSUM bank per destination tile
            call_idx = 0
            psums = {}
            for s in range(n_super):
                ts = list(range(s * stile, min((s + 1) * stile, tiles)))
                while call_idx < len(calls):
                    b, c0, csize = calls[call_idx]
                    if int(sched["chunk_super"][c0]) != s:
                        break
                    qn = 0
                    call_idx += 1
                    idx_tile = meta_pool.tile([P, w * 8], I16, tag="idx")
                    nc.sync.dma_start(
                        idx_tile[:, : csize * 8],
                        idx_t[:, c0 * 8 : (c0 + csize) * 8],
                    )
                    ldst_tile = meta_pool.tile([P, w], BF16, tag="ldst")
                    nc.sync.dma_start(
                        ldst_tile[:, :csize], ldst_t[:, c0 : c0 + csize]
                    )
                    msgs = gather_pool.tile([P, w, elem], BF16)
                    nc.gpsimd.dma_gather(
                        out_ap=msgs[:, :csize, :],
                        in_ap=x_t[b * block : min((b + 1) * block, n_nodes), :],
                        idxs_ap=idx_tile[:, : csize * 8],
                        num_idxs=csize * P,
                        num_idxs_reg=csize * P,
                        elem_size=elem,
                        single_packet=SINGLE_PACKET,
                        queue_num=qn,
                    )
                    for j0 in range(0, csize, 4):
                        g = min(4, csize - j0)
                        onehot = oh_pool.tile([P, 4 * P], BF16, name="oh", tag="oh")
                        lt = ldst_tile[:, j0 : j0 + g]
                        lt_b = bass.AP(lt.tensor, lt.offset, lt.ap + [[0, P]])
                        nc.vector.tensor_tensor(
                            out=onehot[:, : g * P].rearrange(
                                "p (g q) -> p g q", q=P
                            ),
                            in0=iota_b[:, : g * P].rearrange(
                                "p (g q) -> p g q", q=P
                            ),
                            in1=lt_b,
                            op=mybir.AluOpType.is_equal,
                        )
                        for jj in range(g):
                            ci = c0 + j0 + jj
                            t = int(chunk_tile[ci])
                            if chunk_first[ci]:
                                psums[t] = psum_pool.tile(
                                    [P, elem], F32, tag="ps", name=f"ps{t}"
                                )
                            nc.tensor.matmul(
                                psums[t][:, :],
                                lhsT=onehot[:, (jj) * P : (jj + 1) * P],
                                rhs=msgs[:, j0 + jj, :],
                                start=bool(chunk_first[ci]),
                                stop=bool(chunk_last[ci]),
                            )
                # evacuate: stage[:, t*dim:+dim] = psum_hi + psum_lo
                for t in ts:
                    if not tile_has[t]:
                        continue
                    ps = psums.pop(t)
                    nc.scalar.copy(stage[:, t * dim : (t + 1) * dim], ps[:, :dim])
                    nc.vector.tensor_tensor(
                        out=stage[:, t * dim : (t + 1) * dim],
                        in0=stage[:, t * dim : (t + 1) * dim],
                        in1=ps[:, dim:],
                        op=mybir.AluOpType.add,
                    )

            out_view = out_t[:, :].rearrange("(t p) d -> p t d", p=P)
            nc.sync.dma_start(out_view, stage[:])

    nc.compile()
    return nc


def _run(x, edge_index, n_nodes, dim, n_cores, block, w, **run_kwargs):
    sched = _prep(edge_index, n_nodes, n_cores, block, w)
    xp = _pack_x(x)
    nc = _build(n_nodes, dim, block, w, sched)
    in_maps = [
        {"xpack": xp, "idx": sched["idx"][k], "ldst": sched["ldst"][k]}
        for k in range(n_cores)
    ]
    res = run_bass_kernel_spmd(
        nc, in_maps, core_ids=list(range(n_cores)), **run_kwargs
    )
    npc = sched["npc"]
    tiles = sched["tiles"]
    perm = sched["perm"]
    parts = []
    for k in range(n_cores):
        r = res.results[k]["out"].reshape(tiles, P, -1)
        inv = np.empty(tiles, np.int64)
        inv[perm[k]] = np.arange(tiles)
        parts.append(r[inv].reshape(tiles * P, -1)[:npc])
    out = np.concatenate(parts, axis=0)
    return out, res


def kernel(x, edge_index):
    out, _ = _run(
        x, edge_index, N_NODES, DIM, N_CORES, SRC_BLOCK, CHUNKS_PER_CALL
    )
    return out
